# revision 37
# baseline (speedup 1.0000x reference)
"""Tensor-parallel causal self-attention (RoPE) for 8 TRN2 NeuronCores.

Sharding: 16 heads -> 2 heads per core (TP). Each core computes
qkv projection for its heads, RoPE, causal attention (exp/softmax
without max-subtraction -- scores are ~N(0,1)), and its partial
out-projection. The host sums the 8 partial outputs (the all-reduce
equivalent of TP out-projection).

Per-core layouts (host pre-transposes and pre-tiles everything so no
on-device transposes are needed and every DMA is contiguous per
partition):
  xTt   [128, 128, 512] bf16  x^T tiled: [p, tci*16+kb, j], replicated
  wTt   [128, 16, 768]  bf16  w tiled [p, kb, m]; m = [q0,q1,k0,k1,v0,v1]
                              head blocks, q pre-scaled by 1/sqrt(D)
  woT   [256, C]  bf16   W_out columns for this core's heads, transposed
  cos2/sin2 [128, T] f32  RoPE tables duplicated in both halves
  out   [B*T, C]  bf16   partial y (host sums over cores)

Schedule notes:
  - chunk 0 of the qkv projection runs kb-outer (all six output blocks
    accumulate per contraction slice) so the PE starts as soon as the
    first weight/x slices land instead of waiting for the full load.
  - V is produced directly in [tok, d] psum layout (x block stationary,
    wv moving) so no PE transposes are needed before att@v.
  - attention uses the S^T layout; rowsums come from a ones[128,128]
    stationary matmul that broadcasts the sum to all partitions, so
    normalization is rowsum -> reciprocal -> multiply (2 DVE ops).
  - rowsum/O matmuls are causally restricted to the [lo:512] columns of
    each diagonal block; the skipped columns are never read.
"""

import math
import sys

sys.path.insert(0, "/opt/trn_rl_repo")

import numpy as np

import concourse.bass as bass
import concourse.mybir as mybir
import concourse.tile as tile
from concourse import bacc
from concourse.bass import ds
from concourse.bass_utils import run_bass_kernel_spmd

F32 = mybir.dt.float32
F32R = mybir.dt.float32r
BF16 = mybir.dt.bfloat16
EXP = mybir.ActivationFunctionType.Exp

B, T, C = 2, 2048, 2048
NH, D = 16, 128
NCORES, HPC = 8, 2          # heads per core
NTOK = B * T                # 4096
KB = C // 128               # 16 contraction blocks
NTC = NTOK // 512           # 8 token chunks of 512
NTB = NTOK // 128           # 32 token blocks of 128
QB = T // 128               # 16 query blocks per (b,h)


def build():
    nc = bacc.Bacc("TRN2", target_bir_lowering=False, debug=False,
                   num_devices=NCORES)
    # x and w are pre-tiled on the host so every DMA reads contiguous
    # multi-KB segments per partition (strided 1KB-segment loads cost
    # ~8us of descriptor generation per chunk on the HWDGE queue).
    # xTt[p, tci*KB + kb, j] = x[tci*512 + j, kb*128 + p]
    xTt = nc.dram_tensor("xTt", [128, NTC * KB, 512], BF16,
                         kind="ExternalInput")
    # wTt[p, kb, m] = w_local[m, kb*128 + p]
    wTt = nc.dram_tensor("wTt", [128, KB, 3 * HPC * D], BF16,
                         kind="ExternalInput")
    woT = nc.dram_tensor("woT", [HPC * D, C], BF16, kind="ExternalInput")
    cos2 = nc.dram_tensor("cos2", [128, T], F32, kind="ExternalInput")
    sin2 = nc.dram_tensor("sin2", [128, T], F32, kind="ExternalInput")
    out = nc.dram_tensor("out", [NTOK, C], BF16, kind="ExternalOutput")

    with tile.TileContext(nc) as tc:
        with tc.tile_pool(name="const", bufs=1) as constp, \
             tc.tile_pool(name="qk", bufs=1) as qkp, \
             tc.tile_pool(name="v", bufs=1) as vp, \
             tc.tile_pool(name="wo", bufs=1) as wop:
            ones_bf = constp.tile([128, 128], BF16, tag="onesbf")

            # q0,q1,k0,k1 in [d, tok] layout (bf16); v in [tok, d] block
            # layout (bf16): block tb occupies columns [tb*256, tb*256+256)
            # as [v0 | v1]. Tiles are split per batch so batch-0 attention
            # does not pick up a coarse-grained dependency on batch-1's
            # rope/eviction writes.
            qk = [[qkp.tile([128, T], BF16, tag=f"qk{bb}_{i}",
                            name=f"qk{bb}_{i}") for i in range(4)]
                  for bb in range(B)]
            v_sb = [vp.tile([128, QB * 2 * D], BF16, tag=f"v{bb}",
                            name=f"v{bb}") for bb in range(B)]
            wo_sb = [wop.tile([128, C], BF16, tag=f"wo{h}", name=f"wo{h}")
                     for h in range(HPC)]

            # ---------------- phase 1: qkv projection + rope + v
            with tc.tile_pool(name="w", bufs=1) as wp, \
                 tc.tile_pool(name="tab", bufs=1) as tabp, \
                 tc.tile_pool(name="x", bufs=2) as xp, \
                 tc.tile_pool(name="ps1", bufs=8, space="PSUM") as ps1, \
                 tc.tile_pool(name="rtmp", bufs=2) as rtmpp:
                # chunk 0: weights and x slices issued interleaved in kb
                # order, 2 contraction blocks at a time, so the kb-outer
                # matmul schedule below starts after only ~650KB lands.
                xa0 = xp.tile([128, KB, 512], BF16, tag="xa", name="xa0")
                xa1 = xp.tile([128, KB, 512], BF16, tag="xa", name="xa1")
                nc.gpsimd.memset(ones_bf[:], 1.0)
                w_grp = []
                for q in range(8):
                    wg = wp.tile([128, 2, 3 * HPC * D], BF16, tag=f"w{q}",
                                 name=f"wg{q}")
                    # the first two groups ride the low-latency HWDGE
                    # scalar queue (sync is busy issuing the x slices)
                    eng = (nc.scalar if q < 2
                           else nc.gpsimd if q % 2 == 0 else nc.scalar)
                    if q >= 4:
                        # the second half of chunk 0's loads is gated
                        # behind an early weight-group arrival so the
                        # first contraction slices get full HBM bandwidth
                        nc.vector.tensor_copy(wg[0:1, 0, 0:1],
                                              w_grp[1][0:1, 0, 0:1])
                        nc.vector.tensor_copy(xa0[0:1, 2 * q, 0:1],
                                              w_grp[1][0:1, 0, 0:1])
                    eng.dma_start(wg[:], wTt[:, ds(2 * q, 2), :])
                    nc.sync.dma_start(xa0[:, ds(2 * q, 2), :],
                                      xTt[:, ds(2 * q, 2), :])
                    w_grp.append(wg)
                w_sb = [w_grp[kb // 2][:, kb % 2, :] for kb in range(KB)]
                # warm the PE HAM clock gate (cold PE runs at 1.2GHz for the
                # first ~3.4us) with dummy matmuls while the first weight
                # and x slices stream in
                warm_ps = ps1.tile([128, 512], F32, tag="qkvps", name="warm")
                for i in range(40):
                    nc.tensor.matmul(warm_ps[:, 0:128], ones_bf[:],
                                     ones_bf[:], start=True, stop=True)
                cos_sb = tabp.tile([128, T], F32, tag="cos", name="cos")
                sin_sb = tabp.tile([128, T], F32, tag="sin", name="sin")

                def rope(mb, psum, tci, c0=0, w=512):
                    # dst_lo = t1*cos - t2*sin, dst_hi = t1*sin + t2*cos
                    s = ds((tci % 4) * 512 + c0, w)
                    pc = ds(c0, w)
                    dst = qk[tci // 4][mb]
                    tmp = rtmpp.tile([128, 512], BF16, tag="ropetmp",
                                     name=f"rt{tci}_{mb}_{c0}")
                    nc.vector.tensor_mul(
                        tmp[0:64, 0:w], psum[64:128, pc], sin_sb[0:64, s])
                    nc.vector.tensor_mul(
                        tmp[64:128, 0:w], psum[0:64, pc], sin_sb[64:128, s])
                    nc.vector.tensor_mul(dst[:, s], psum[:, pc],
                                         cos_sb[:, s])
                    nc.vector.tensor_sub(
                        dst[0:64, s], dst[0:64, s], tmp[0:64, 0:w])
                    nc.vector.tensor_add(
                        dst[64:128, s], dst[64:128, s], tmp[64:128, 0:w])

                for tci in range(NTC):
                    if tci == 0:
                        xa = xa0
                    elif tci == 1:
                        xa = xa1
                    else:
                        xa = xp.tile([128, KB, 512], BF16, tag="xa",
                                     name=f"xa{tci}")
                        nc.sync.dma_start(xa[:], xTt[:, ds(tci * KB, KB), :])

                    # psums: 4 qk blocks [d, tok] + 2 v banks, each packing
                    # two [tok, 256] blocks side by side
                    psq = [ps1.tile([128, 512], F32, tag="qkvps",
                                    name=f"psq{tci}_{mb}") for mb in range(4)]
                    psv = [ps1.tile([128, 512], F32, tag="qkvps",
                                    name=f"psv{tci}_{j}") for j in range(2)]

                    def qk_mm(mb, kb):
                        nc.tensor.matmul(
                            psq[mb][:], w_sb[kb][:, ds(mb * 128, 128)],
                            xa[:, kb, :], start=(kb == 0), stop=(kb == KB - 1))

                    def v_mm(tb, kb):
                        # start=True clears has_written bits for the WHOLE
                        # bank, so only the first group touching a bank may
                        # set it; the second group's first write lands on
                        # clear bits and overwrites (HW accumulate rules).
                        nc.tensor.matmul(
                            psv[tb // 2][:, ds((tb % 2) * 256, 256)],
                            xa[:, kb, ds(tb * 128, 128)],
                            w_sb[kb][:, ds(4 * 128, 256)],
                            start=(kb == 0 and tb % 2 == 0),
                            stop=(kb == KB - 1), skip_group_check=True)

                    if tci == 0:
                        # kb-outer: consume each contraction slice across
                        # all six outputs as soon as its DMA lands
                        for kb in range(KB):
                            for mb in range(4):
                                qk_mm(mb, kb)
                            for tb in range(4):
                                v_mm(tb, kb)
                        # secondary loads (rope tables, wo, chunk-1 x) are
                        # gated behind mid-stream weight-group arrivals
                        # with tiny dependency copies so ~5MB of DMA does
                        # not steal HBM bandwidth from the startup-critical
                        # w/x stream
                        for gdst, wgate in ((cos_sb[0:1, 0:1], 3),
                                            (sin_sb[0:1, 0:1], 3),
                                            (wo_sb[0][0:1, 0:1], 6),
                                            (wo_sb[1][0:1, 0:1], 6),
                                            (xa1[0:1, 0, 0:1], 6)):
                            nc.vector.tensor_copy(
                                gdst, w_grp[wgate][0:1, 0, 0:1])
                        nc.gpsimd.dma_start(cos_sb[:], cos2[:])
                        nc.gpsimd.dma_start(sin_sb[:], sin2[:])
                        for h in range(HPC):
                            nc.scalar.dma_start(wo_sb[h][:],
                                                woT[ds(h * 128, 128), :])
                        nc.sync.dma_start(xa1[:], xTt[:, ds(KB, KB), :])
                    else:
                        # mb-outer, evictions emitted right after each
                        # group's stop so the DVE/scalar drain tracks the
                        # PE instead of piling up at chunk end
                        for mb in (0, 4, 1, 2, 5, 3):
                            if mb < 4:
                                if tci == NTC - 1 and mb == 3:
                                    # final group of phase 1 split into two
                                    # column halves so the phase-boundary
                                    # pool barrier waits on a half-size
                                    # rope drain
                                    for half in range(2):
                                        cw = ds(half * 256, 256)
                                        for kb in range(KB):
                                            nc.tensor.matmul(
                                                psq[mb][:, cw],
                                                w_sb[kb][:, ds(mb * 128,
                                                               128)],
                                                xa[:, kb, cw],
                                                start=(kb == 0 and
                                                       half == 0),
                                                stop=(kb == KB - 1),
                                                skip_group_check=True)
                                        rope(mb, psq[mb], tci,
                                             half * 256, 256)
                                    continue
                                for kb in range(KB):
                                    qk_mm(mb, kb)
                                rope(mb, psq[mb], tci)
                            else:
                                j = 0 if mb == 4 else 1
                                for tb in (0, 1) if mb == 4 else (2, 3):
                                    for kb in range(KB):
                                        v_mm(tb, kb)
                                nc.scalar.copy(
                                    v_sb[tci // 4][:, ds(
                                        ((tci % 4) * 4 + 2 * j) * 256, 512)],
                                    psv[j][:])
                    if tci == 0:
                        for mb in range(4):
                            rope(mb, psq[mb], tci)
                        for j in range(2):
                            nc.scalar.copy(
                                v_sb[tci // 4][:, ds(
                                    ((tci % 4) * 4 + 2 * j) * 256, 512)],
                                psv[j][:])

            # ---------------- phase 2: attention per (b, h), transposed
            # layout: S^T[tk, tq] = matmul(lhsT=kT block, rhs=qT chunk), so
            # softmax normalization happens on the output columns and P^T
            # feeds att@v directly. Rowsums come from a ones[128,128]
            # matmul over P^T (broadcast to all partitions).
            otp = tc.alloc_tile_pool(name="ot", bufs=1)
            ot_sb = [otp.tile([128, NTOK], BF16, tag=f"ot{h}", name=f"ot{h}")
                     for h in range(HPC)]
            with tc.tile_pool(name="sps", bufs=3, space="PSUM") as sps, \
                 tc.tile_pool(name="pt", bufs=10) as ptp_pool, \
                 tc.tile_pool(name="rsps", bufs=1, space="PSUM") as rsps, \
                 tc.tile_pool(name="rbc", bufs=2) as rbcp, \
                 tc.tile_pool(name="qs", bufs=2) as qsump, \
                 tc.tile_pool(name="ops", bufs=2, space="PSUM") as opsp, \
                 tc.tile_pool(name="yps", bufs=2, space="PSUM") as ypsp, \
                 tc.tile_pool(name="y", bufs=2) as ypool:
                # groups in (b, g, h) order; the first 3 S matmuls of
                # group k+1 are emitted near the end of group k so their
                # exps run under the preceding PE work and the group
                # boundary never waits on exp latency
                groups = [(b, g, h) for b in range(B)
                          for g in range(4) for h in range(HPC)]
                gstate = {}

                def prefetch(idx):
                    if idx >= len(groups) or idx in gstate:
                        return
                    gb, gg, gh = groups[idx]
                    qT, kT = qk[gb][gh], qk[gb][2 + gh]
                    pts = {}

                    def emit_s(i):
                        lo = max(i - 4 * gg, 0) * 128
                        n = 512 - lo
                        st = sps.tile([128, 512], F32, tag="s")
                        nc.tensor.matmul(
                            st[:, ds(lo, n)],
                            kT[:, ds(i * 128, 128)],
                            qT[:, ds(gg * 512 + lo, n)],
                            start=True, stop=True)
                        pt = ptp_pool.tile([128, 512], BF16, tag="pt")
                        nc.scalar.activation(
                            pt[:, ds(lo, n)], st[:, ds(lo, n)], EXP)
                        if i >= 4 * gg:
                            # zero the non-causal upper triangle of the
                            # diagonal block post-exp
                            nc.gpsimd.affine_select(
                                out=pt[:, ds(lo, 128)],
                                in_=pt[:, ds(lo, 128)],
                                compare_op=mybir.AluOpType.is_ge,
                                fill=0.0, base=0, pattern=[[1, 128]],
                                channel_multiplier=-1)
                        pts[i] = (pt, lo, n)

                    gstate[idx] = (pts, emit_s)
                    for i in range(min(3, 4 * gg + 4)):
                        emit_s(i)

                # the out-projection of each chunk is not emitted as one
                # block: its matmul pairs are queued and dripped into the
                # FOLLOWING attention groups' loops, filling the PE slots
                # that otherwise idle on exp throughput
                pending_op = []

                def mk_pair(tkb, pair, hold, last_tkb, last_grp):
                    def job():
                        if pair == 0:
                            hold[0] = ypool.tile([128, C], BF16, tag="y",
                                                 name=f"ysb{tkb}")
                        ysb = hold[0]
                        yp2 = [ypsp.tile([128, 512], F32, tag="yp",
                                         name=f"yp{tkb}_{pair}_{oc}")
                               for oc in range(2)]
                        for h2 in range(HPC):
                            for oc in range(2):
                                nc.tensor.matmul(
                                    yp2[oc][:],
                                    ot_sb[h2][:, ds(tkb * 128, 128)],
                                    wo_sb[h2][:, ds((pair * 2 + oc) * 512,
                                                    512)],
                                    start=(h2 == 0), stop=(h2 == HPC - 1))
                        for oc in range(2):
                            col = ds((pair * 2 + oc) * 512, 512)
                            if oc == 0:
                                nc.scalar.copy(ysb[:, col], yp2[oc][:])
                            else:
                                nc.vector.tensor_copy(ysb[:, col],
                                                      yp2[oc][:])
                        if last_tkb:
                            # DMA each half as soon as it is ready
                            nc.sync.dma_start(
                                out[ds(tkb * 128, 128),
                                    ds(pair * 1024, 1024)],
                                ysb[:, ds(pair * 1024, 1024)])
                        elif pair == 1:
                            # the final group keeps everything on the HWDGE
                            # queue so the SWDGE drain at kernel end has
                            # nothing left to wait for
                            oeng = (nc.sync if (tkb % 2 == 0 or last_grp)
                                    else nc.gpsimd)
                            oeng.dma_start(out[ds(tkb * 128, 128), :],
                                           ysb[:])
                    return job

                def queue_outproj(qb, qg):
                    last_grp = (qb == B - 1 and qg == 3)
                    for tkl in range(4 * qg, 4 * qg + 4):
                        tkb = qb * QB + tkl
                        last_tkb = last_grp and tkl == 4 * qg + 3
                        hold = [None]
                        for pair in range(2):
                            pending_op.append(
                                mk_pair(tkb, pair, hold, last_tkb,
                                        last_grp))

                prefetch(0)
                for idx, (b, g, h) in enumerate(groups):
                    base = b * T
                    ntk = 4 * g + 4
                    pts, emit_s = gstate.pop(idx)
                    o_ps = opsp.tile([128, 512], F32, tag="o")
                    rs_ps = rsps.tile([128, 512], F32, tag="rs")
                    first_rs = [True]
                    qsums = {}

                    def rs_mm(src, lo, n, stop):
                        nc.tensor.matmul(
                            rs_ps[:, ds(lo, n)], ones_bf[:], src,
                            start=first_rs[0], stop=stop,
                            skip_group_check=True)
                        first_rs[0] = False

                    def emit_o(i):
                        pt, lo, n = pts[i]
                        nc.tensor.matmul(
                            o_ps[:, ds(lo, n)],
                            v_sb[b][:, ds(i * 256 + h * 128, 128)],
                            pt[:, ds(lo, n)],
                            start=(i == 0), stop=(i == ntk - 1),
                            skip_group_check=True)

                    # rowsums of full-width (sub-diagonal) blocks are
                    # batched: 4 P tiles summed on the DVE, one rowsum
                    # matmul instead of 4; the matmul trails the sum by
                    # ~2 blocks so the PE never waits on it.
                    for i in range(ntk):
                        if i + 3 < ntk:
                            emit_s(i + 3)
                        if i == ntk - 2:
                            prefetch(idx + 1)
                        if i % 4 == 3 and i < 4 * g:
                            q = i // 4
                            qs = qsump.tile([128, 512], BF16, tag="qs",
                                            name=f"qs{b}{g}{h}{q}")
                            nc.vector.tensor_add(
                                qs[:], pts[4 * q][0][:],
                                pts[4 * q + 1][0][:])
                            nc.vector.tensor_add(
                                qs[:], qs[:], pts[4 * q + 2][0][:])
                            nc.vector.tensor_add(
                                qs[:], qs[:], pts[4 * q + 3][0][:])
                            qsums[q] = qs
                        if i >= 6 and (i - 6) % 4 == 0 \
                                and (i - 6) // 4 < g:
                            rs_mm(qsums[(i - 6) // 4][:], 0, 512, False)
                        emit_o(i)
                        if i >= 4 * g:
                            pt, lo, n = pts[i]
                            rs_mm(pt[:, ds(lo, n)], lo, n, i == ntk - 1)
                        # every other block, so the fill work stretches
                        # across the whole exp-bound span of long groups
                        if pending_op and i % 2 == 1:
                            pending_op.pop(0)()

                    rbc = rbcp.tile([128, 512], F32, tag="rbc")
                    nc.vector.reciprocal_approx_fast(rbc[:], rs_ps[:])
                    nc.vector.tensor_mul(
                        ot_sb[h][:, ds(base + g * 512, 512)],
                        o_ps[:], rbc[:])

                    if h == HPC - 1:
                        queue_outproj(b, g)
                while pending_op:
                    pending_op.pop(0)()
            otp.release()

    nc.compile()
    return nc


_NC_CACHE = []


def _get_nc():
    if not _NC_CACHE:
        _NC_CACHE.append(build())
    return _NC_CACHE[0]


def make_in_maps(x, w_qkv, w_out):
    import ml_dtypes

    x2 = x.reshape(NTOK, C).astype(np.float32)
    # xTt[p, tci*KB + kb, j] = x[tci*512 + j, kb*128 + p]
    xTt = np.ascontiguousarray(
        x2.reshape(NTC, 512, KB, 128).transpose(3, 0, 2, 1)
        .reshape(128, NTC * KB, 512)).astype(ml_dtypes.bfloat16)
    scale = np.float32(1.0 / math.sqrt(D))

    inv = 1.0 / (10000.0 ** (np.arange(0, D, 2, dtype=np.float32) / D))
    pos = np.arange(T, dtype=np.float32)
    ang = pos[:, None] * inv[None, :]            # [T, 64]
    cosT = np.cos(ang).T.astype(np.float32)      # [64, T]
    sinT = np.sin(ang).T.astype(np.float32)
    cos2 = np.ascontiguousarray(np.vstack([cosT, cosT]))   # [128, T]
    sin2 = np.ascontiguousarray(np.vstack([sinT, sinT]))

    in_maps = []
    for c in range(NCORES):
        q = w_qkv[256 * c: 256 * (c + 1)] * scale
        k = w_qkv[C + 256 * c: C + 256 * (c + 1)]
        v = w_qkv[2 * C + 256 * c: 2 * C + 256 * (c + 1)]
        wl = np.concatenate([q, k, v], axis=0)       # [768, C]
        # wTt[p, kb, m] = wl[m, kb*128 + p]
        wTt = np.ascontiguousarray(
            wl.T.reshape(KB, 128, 3 * HPC * D).transpose(1, 0, 2)
        ).astype(ml_dtypes.bfloat16)
        woT = np.ascontiguousarray(
            w_out[:, 256 * c: 256 * (c + 1)].T).astype(ml_dtypes.bfloat16)
        in_maps.append({
            "xTt": xTt, "wTt": wTt, "woT": woT,
            "cos2": cos2, "sin2": sin2,
        })
    return in_maps


def run(x, w_qkv, w_out, trace=False):
    nc = _get_nc()
    in_maps = make_in_maps(x, w_qkv, w_out)
    res = run_bass_kernel_spmd(nc, in_maps, core_ids=list(range(NCORES)),
                               trace=trace)
    y = res.results[0]["out"].astype(np.float32)
    for i in range(1, NCORES):
        y = y + res.results[i]["out"].astype(np.float32)
    return y.reshape(B, T, C), res


def kernel(x, w_qkv, w_out):
    y, _ = run(x, w_qkv, w_out, trace=False)
    return y


# revision 38
# speedup vs baseline: 1.0173x; 1.0173x over previous
"""Tensor-parallel causal self-attention (RoPE) for 8 TRN2 NeuronCores.

Sharding: 16 heads -> 2 heads per core (TP). Each core computes
qkv projection for its heads, RoPE, causal attention (exp/softmax
without max-subtraction -- scores are ~N(0,1)), and its partial
out-projection. The host sums the 8 partial outputs (the all-reduce
equivalent of TP out-projection).

Per-core layouts (host pre-transposes and pre-tiles everything so no
on-device transposes are needed and every DMA is contiguous per
partition):
  xTt   [128, 128, 512] bf16  x^T tiled: [p, tci*16+kb, j], replicated
  wTt   [128, 16, 768]  bf16  w tiled [p, kb, m]; m = [q0,q1,k0,k1,v0,v1]
                              head blocks, q pre-scaled by 1/sqrt(D)
  woT   [256, C]  bf16   W_out columns for this core's heads, transposed
  cos2/sin2 [128, T] f32  RoPE tables duplicated in both halves
  out   [B*T, C]  bf16   partial y (host sums over cores)

Schedule notes:
  - chunk 0 of the qkv projection runs kb-outer (all six output blocks
    accumulate per contraction slice) so the PE starts as soon as the
    first weight/x slices land instead of waiting for the full load.
  - V is produced directly in [tok, d] psum layout (x block stationary,
    wv moving) so no PE transposes are needed before att@v.
  - attention uses the S^T layout; rowsums come from a ones[128,128]
    stationary matmul that broadcasts the sum to all partitions, so
    normalization is rowsum -> reciprocal -> multiply (2 DVE ops).
  - rowsum/O matmuls are causally restricted to the [lo:512] columns of
    each diagonal block; the skipped columns are never read.
"""

import math
import sys

sys.path.insert(0, "/opt/trn_rl_repo")

import numpy as np

import concourse.bass as bass
import concourse.mybir as mybir
import concourse.tile as tile
from concourse import bacc
from concourse.bass import ds
from concourse.bass_utils import run_bass_kernel_spmd

F32 = mybir.dt.float32
F32R = mybir.dt.float32r
BF16 = mybir.dt.bfloat16
EXP = mybir.ActivationFunctionType.Exp

B, T, C = 2, 2048, 2048
NH, D = 16, 128
NCORES, HPC = 8, 2          # heads per core
NTOK = B * T                # 4096
KB = C // 128               # 16 contraction blocks
NTC = NTOK // 512           # 8 token chunks of 512
NTB = NTOK // 128           # 32 token blocks of 128
QB = T // 128               # 16 query blocks per (b,h)


def build():
    nc = bacc.Bacc("TRN2", target_bir_lowering=False, debug=False,
                   num_devices=NCORES)
    # x and w are pre-tiled on the host so every DMA reads contiguous
    # multi-KB segments per partition (strided 1KB-segment loads cost
    # ~8us of descriptor generation per chunk on the HWDGE queue).
    # xTt[p, tci*KB + kb, j] = x[tci*512 + j, kb*128 + p]
    xTt = nc.dram_tensor("xTt", [128, NTC * KB, 512], BF16,
                         kind="ExternalInput")
    # wTt[p, kb, m] = w_local[m, kb*128 + p]
    wTt = nc.dram_tensor("wTt", [128, KB, 3 * HPC * D], BF16,
                         kind="ExternalInput")
    woT = nc.dram_tensor("woT", [HPC * D, C], BF16, kind="ExternalInput")
    cos2 = nc.dram_tensor("cos2", [128, T], F32, kind="ExternalInput")
    sin2 = nc.dram_tensor("sin2", [128, T], F32, kind="ExternalInput")
    out = nc.dram_tensor("out", [NTOK, C], BF16, kind="ExternalOutput")

    with tile.TileContext(nc) as tc:
        with tc.tile_pool(name="const", bufs=1) as constp, \
             tc.tile_pool(name="qk", bufs=1) as qkp, \
             tc.tile_pool(name="v", bufs=1) as vp, \
             tc.tile_pool(name="wo", bufs=1) as wop:
            ones_bf = constp.tile([128, 128], BF16, tag="onesbf")

            # q0,q1,k0,k1 in [d, tok] layout (bf16); v in [tok, d] block
            # layout (bf16): block tb occupies columns [tb*256, tb*256+256)
            # as [v0 | v1]. Tiles are split per batch so batch-0 attention
            # does not pick up a coarse-grained dependency on batch-1's
            # rope/eviction writes.
            qk = [[qkp.tile([128, T], BF16, tag=f"qk{bb}_{i}",
                            name=f"qk{bb}_{i}") for i in range(4)]
                  for bb in range(B)]
            v_sb = [vp.tile([128, QB * 2 * D], BF16, tag=f"v{bb}",
                            name=f"v{bb}") for bb in range(B)]
            wo_sb = [wop.tile([128, C], BF16, tag=f"wo{h}", name=f"wo{h}")
                     for h in range(HPC)]

            # ---------------- phase 1: qkv projection + rope + v
            with tc.tile_pool(name="w", bufs=1) as wp, \
                 tc.tile_pool(name="tab", bufs=1) as tabp, \
                 tc.tile_pool(name="x", bufs=2) as xp, \
                 tc.tile_pool(name="ps1", bufs=8, space="PSUM") as ps1, \
                 tc.tile_pool(name="rtmp", bufs=2) as rtmpp:
                # chunk 0: weights and x slices issued interleaved in kb
                # order, 2 contraction blocks at a time, so the kb-outer
                # matmul schedule below starts after only ~650KB lands.
                xa0 = xp.tile([128, KB, 512], BF16, tag="xa", name="xa0")
                xa1 = xp.tile([128, KB, 512], BF16, tag="xa", name="xa1")
                nc.gpsimd.memset(ones_bf[:], 1.0)
                w_grp = []
                for q in range(8):
                    wg = wp.tile([128, 2, 3 * HPC * D], BF16, tag=f"w{q}",
                                 name=f"wg{q}")
                    # the first two groups ride the low-latency HWDGE
                    # scalar queue (sync is busy issuing the x slices)
                    eng = (nc.scalar if q < 2
                           else nc.gpsimd if q % 2 == 0 else nc.scalar)
                    eng.dma_start(wg[:], wTt[:, ds(2 * q, 2), :])
                    nc.sync.dma_start(xa0[:, ds(2 * q, 2), :],
                                      xTt[:, ds(2 * q, 2), :])
                    w_grp.append(wg)
                w_sb = [w_grp[kb // 2][:, kb % 2, :] for kb in range(KB)]
                # warm the PE HAM clock gate (cold PE runs at 1.2GHz for the
                # first ~3.4us) with dummy matmuls while the first weight
                # and x slices stream in
                warm_ps = ps1.tile([128, 512], F32, tag="qkvps", name="warm")
                for i in range(40):
                    nc.tensor.matmul(warm_ps[:, 0:128], ones_bf[:],
                                     ones_bf[:], start=True, stop=True)
                cos_sb = tabp.tile([128, T], F32, tag="cos", name="cos")
                sin_sb = tabp.tile([128, T], F32, tag="sin", name="sin")

                def rope(mb, psum, tci, c0=0, w=512):
                    # dst_lo = t1*cos - t2*sin, dst_hi = t1*sin + t2*cos
                    s = ds((tci % 4) * 512 + c0, w)
                    pc = ds(c0, w)
                    dst = qk[tci // 4][mb]
                    tmp = rtmpp.tile([128, 512], BF16, tag="ropetmp",
                                     name=f"rt{tci}_{mb}_{c0}")
                    nc.vector.tensor_mul(
                        tmp[0:64, 0:w], psum[64:128, pc], sin_sb[0:64, s])
                    nc.vector.tensor_mul(
                        tmp[64:128, 0:w], psum[0:64, pc], sin_sb[64:128, s])
                    nc.vector.tensor_mul(dst[:, s], psum[:, pc],
                                         cos_sb[:, s])
                    nc.vector.tensor_sub(
                        dst[0:64, s], dst[0:64, s], tmp[0:64, 0:w])
                    nc.vector.tensor_add(
                        dst[64:128, s], dst[64:128, s], tmp[64:128, 0:w])

                for tci in range(NTC):
                    if tci == 0:
                        xa = xa0
                    elif tci == 1:
                        xa = xa1
                    else:
                        xa = xp.tile([128, KB, 512], BF16, tag="xa",
                                     name=f"xa{tci}")
                        nc.sync.dma_start(xa[:], xTt[:, ds(tci * KB, KB), :])

                    # psums: 4 qk blocks [d, tok] + 2 v banks, each packing
                    # two [tok, 256] blocks side by side
                    psq = [ps1.tile([128, 512], F32, tag="qkvps",
                                    name=f"psq{tci}_{mb}") for mb in range(4)]
                    psv = [ps1.tile([128, 512], F32, tag="qkvps",
                                    name=f"psv{tci}_{j}") for j in range(2)]

                    def qk_mm(mb, kb):
                        nc.tensor.matmul(
                            psq[mb][:], w_sb[kb][:, ds(mb * 128, 128)],
                            xa[:, kb, :], start=(kb == 0), stop=(kb == KB - 1))

                    def v_mm(tb, kb):
                        # start=True clears has_written bits for the WHOLE
                        # bank, so only the first group touching a bank may
                        # set it; the second group's first write lands on
                        # clear bits and overwrites (HW accumulate rules).
                        nc.tensor.matmul(
                            psv[tb // 2][:, ds((tb % 2) * 256, 256)],
                            xa[:, kb, ds(tb * 128, 128)],
                            w_sb[kb][:, ds(4 * 128, 256)],
                            start=(kb == 0 and tb % 2 == 0),
                            stop=(kb == KB - 1), skip_group_check=True)

                    if tci == 0:
                        # kb-outer: consume each contraction slice across
                        # all six outputs as soon as its DMA lands
                        for kb in range(KB):
                            for mb in range(4):
                                qk_mm(mb, kb)
                            for tb in range(4):
                                v_mm(tb, kb)
                        # secondary loads (rope tables, wo, chunk-1 x) are
                        # gated behind mid-stream weight-group arrivals
                        # with tiny dependency copies so ~5MB of DMA does
                        # not steal HBM bandwidth from the startup-critical
                        # w/x stream
                        for gdst, wgate in ((cos_sb[0:1, 0:1], 3),
                                            (sin_sb[0:1, 0:1], 3),
                                            (wo_sb[0][0:1, 0:1], 6),
                                            (wo_sb[1][0:1, 0:1], 6),
                                            (xa1[0:1, 0, 0:1], 6)):
                            nc.vector.tensor_copy(
                                gdst, w_grp[wgate][0:1, 0, 0:1])
                        nc.gpsimd.dma_start(cos_sb[:], cos2[:])
                        nc.gpsimd.dma_start(sin_sb[:], sin2[:])
                        for h in range(HPC):
                            nc.scalar.dma_start(wo_sb[h][:],
                                                woT[ds(h * 128, 128), :])
                        nc.sync.dma_start(xa1[:], xTt[:, ds(KB, KB), :])
                    else:
                        # mb-outer, evictions emitted right after each
                        # group's stop so the DVE/scalar drain tracks the
                        # PE instead of piling up at chunk end
                        for mb in (0, 4, 1, 2, 5, 3):
                            if mb < 4:
                                if tci == NTC - 1 and mb == 3:
                                    # final group of phase 1 split into two
                                    # column halves so the phase-boundary
                                    # pool barrier waits on a half-size
                                    # rope drain
                                    for half in range(2):
                                        cw = ds(half * 256, 256)
                                        for kb in range(KB):
                                            nc.tensor.matmul(
                                                psq[mb][:, cw],
                                                w_sb[kb][:, ds(mb * 128,
                                                               128)],
                                                xa[:, kb, cw],
                                                start=(kb == 0 and
                                                       half == 0),
                                                stop=(kb == KB - 1),
                                                skip_group_check=True)
                                        rope(mb, psq[mb], tci,
                                             half * 256, 256)
                                    continue
                                for kb in range(KB):
                                    qk_mm(mb, kb)
                                rope(mb, psq[mb], tci)
                            else:
                                j = 0 if mb == 4 else 1
                                for tb in (0, 1) if mb == 4 else (2, 3):
                                    for kb in range(KB):
                                        v_mm(tb, kb)
                                nc.scalar.copy(
                                    v_sb[tci // 4][:, ds(
                                        ((tci % 4) * 4 + 2 * j) * 256, 512)],
                                    psv[j][:])
                    if tci == 0:
                        for mb in range(4):
                            rope(mb, psq[mb], tci)
                        for j in range(2):
                            nc.scalar.copy(
                                v_sb[tci // 4][:, ds(
                                    ((tci % 4) * 4 + 2 * j) * 256, 512)],
                                psv[j][:])

            # ---------------- phase 2: attention per (b, h), transposed
            # layout: S^T[tk, tq] = matmul(lhsT=kT block, rhs=qT chunk), so
            # softmax normalization happens on the output columns and P^T
            # feeds att@v directly. Rowsums come from a ones[128,128]
            # matmul over P^T (broadcast to all partitions).
            otp = tc.alloc_tile_pool(name="ot", bufs=1)
            ot_sb = [otp.tile([128, NTOK], BF16, tag=f"ot{h}", name=f"ot{h}")
                     for h in range(HPC)]
            with tc.tile_pool(name="sps", bufs=3, space="PSUM") as sps, \
                 tc.tile_pool(name="pt", bufs=10) as ptp_pool, \
                 tc.tile_pool(name="rsps", bufs=1, space="PSUM") as rsps, \
                 tc.tile_pool(name="rbc", bufs=2) as rbcp, \
                 tc.tile_pool(name="qs", bufs=2) as qsump, \
                 tc.tile_pool(name="ops", bufs=2, space="PSUM") as opsp, \
                 tc.tile_pool(name="yps", bufs=2, space="PSUM") as ypsp, \
                 tc.tile_pool(name="y", bufs=2) as ypool:
                # groups in (b, g, h) order; the first 3 S matmuls of
                # group k+1 are emitted near the end of group k so their
                # exps run under the preceding PE work and the group
                # boundary never waits on exp latency
                groups = [(b, g, h) for b in range(B)
                          for g in range(4) for h in range(HPC)]
                gstate = {}

                def prefetch(idx):
                    if idx >= len(groups) or idx in gstate:
                        return
                    gb, gg, gh = groups[idx]
                    qT, kT = qk[gb][gh], qk[gb][2 + gh]
                    pts = {}

                    def emit_s(i):
                        lo = max(i - 4 * gg, 0) * 128
                        n = 512 - lo
                        st = sps.tile([128, 512], F32, tag="s")
                        nc.tensor.matmul(
                            st[:, ds(lo, n)],
                            kT[:, ds(i * 128, 128)],
                            qT[:, ds(gg * 512 + lo, n)],
                            start=True, stop=True)
                        pt = ptp_pool.tile([128, 512], BF16, tag="pt")
                        nc.scalar.activation(
                            pt[:, ds(lo, n)], st[:, ds(lo, n)], EXP)
                        if i >= 4 * gg:
                            # zero the non-causal upper triangle of the
                            # diagonal block post-exp
                            nc.gpsimd.affine_select(
                                out=pt[:, ds(lo, 128)],
                                in_=pt[:, ds(lo, 128)],
                                compare_op=mybir.AluOpType.is_ge,
                                fill=0.0, base=0, pattern=[[1, 128]],
                                channel_multiplier=-1)
                        pts[i] = (pt, lo, n)

                    gstate[idx] = (pts, emit_s)
                    for i in range(min(3, 4 * gg + 4)):
                        emit_s(i)

                # the out-projection of each chunk is not emitted as one
                # block: its matmul pairs are queued and dripped into the
                # FOLLOWING attention groups' loops, filling the PE slots
                # that otherwise idle on exp throughput
                pending_op = []

                def mk_pair(tkb, pair, hold, last_tkb, last_grp):
                    def job():
                        if pair == 0:
                            hold[0] = ypool.tile([128, C], BF16, tag="y",
                                                 name=f"ysb{tkb}")
                        ysb = hold[0]
                        yp2 = [ypsp.tile([128, 512], F32, tag="yp",
                                         name=f"yp{tkb}_{pair}_{oc}")
                               for oc in range(2)]
                        for h2 in range(HPC):
                            for oc in range(2):
                                nc.tensor.matmul(
                                    yp2[oc][:],
                                    ot_sb[h2][:, ds(tkb * 128, 128)],
                                    wo_sb[h2][:, ds((pair * 2 + oc) * 512,
                                                    512)],
                                    start=(h2 == 0), stop=(h2 == HPC - 1))
                        for oc in range(2):
                            col = ds((pair * 2 + oc) * 512, 512)
                            if oc == 0:
                                nc.scalar.copy(ysb[:, col], yp2[oc][:])
                            else:
                                nc.vector.tensor_copy(ysb[:, col],
                                                      yp2[oc][:])
                        if last_tkb:
                            # DMA each half as soon as it is ready
                            nc.sync.dma_start(
                                out[ds(tkb * 128, 128),
                                    ds(pair * 1024, 1024)],
                                ysb[:, ds(pair * 1024, 1024)])
                        elif pair == 1:
                            # the final group keeps everything on the HWDGE
                            # queue so the SWDGE drain at kernel end has
                            # nothing left to wait for
                            oeng = (nc.sync if (tkb % 2 == 0 or last_grp)
                                    else nc.gpsimd)
                            oeng.dma_start(out[ds(tkb * 128, 128), :],
                                           ysb[:])
                    return job

                def queue_outproj(qb, qg):
                    last_grp = (qb == B - 1 and qg == 3)
                    for tkl in range(4 * qg, 4 * qg + 4):
                        tkb = qb * QB + tkl
                        last_tkb = last_grp and tkl == 4 * qg + 3
                        hold = [None]
                        for pair in range(2):
                            pending_op.append(
                                mk_pair(tkb, pair, hold, last_tkb,
                                        last_grp))

                prefetch(0)
                for idx, (b, g, h) in enumerate(groups):
                    base = b * T
                    ntk = 4 * g + 4
                    pts, emit_s = gstate.pop(idx)
                    o_ps = opsp.tile([128, 512], F32, tag="o")
                    rs_ps = rsps.tile([128, 512], F32, tag="rs")
                    first_rs = [True]
                    qsums = {}

                    def rs_mm(src, lo, n, stop):
                        nc.tensor.matmul(
                            rs_ps[:, ds(lo, n)], ones_bf[:], src,
                            start=first_rs[0], stop=stop,
                            skip_group_check=True)
                        first_rs[0] = False

                    def emit_o(i):
                        pt, lo, n = pts[i]
                        nc.tensor.matmul(
                            o_ps[:, ds(lo, n)],
                            v_sb[b][:, ds(i * 256 + h * 128, 128)],
                            pt[:, ds(lo, n)],
                            start=(i == 0), stop=(i == ntk - 1),
                            skip_group_check=True)

                    # rowsums of full-width (sub-diagonal) blocks are
                    # batched: 4 P tiles summed on the DVE, one rowsum
                    # matmul instead of 4; the matmul trails the sum by
                    # ~2 blocks so the PE never waits on it.
                    for i in range(ntk):
                        if i + 3 < ntk:
                            emit_s(i + 3)
                        if i == ntk - 2:
                            prefetch(idx + 1)
                        if i % 4 == 3 and i < 4 * g:
                            q = i // 4
                            qs = qsump.tile([128, 512], BF16, tag="qs",
                                            name=f"qs{b}{g}{h}{q}")
                            nc.vector.tensor_add(
                                qs[:], pts[4 * q][0][:],
                                pts[4 * q + 1][0][:])
                            nc.vector.tensor_add(
                                qs[:], qs[:], pts[4 * q + 2][0][:])
                            nc.vector.tensor_add(
                                qs[:], qs[:], pts[4 * q + 3][0][:])
                            qsums[q] = qs
                        if i >= 6 and (i - 6) % 4 == 0 \
                                and (i - 6) // 4 < g:
                            rs_mm(qsums[(i - 6) // 4][:], 0, 512, False)
                        emit_o(i)
                        if i >= 4 * g:
                            pt, lo, n = pts[i]
                            rs_mm(pt[:, ds(lo, n)], lo, n, i == ntk - 1)
                        if pending_op and i >= 1:
                            pending_op.pop(0)()

                    rbc = rbcp.tile([128, 512], F32, tag="rbc")
                    nc.vector.reciprocal_approx_fast(rbc[:], rs_ps[:])
                    nc.vector.tensor_mul(
                        ot_sb[h][:, ds(base + g * 512, 512)],
                        o_ps[:], rbc[:])

                    if h == HPC - 1:
                        queue_outproj(b, g)
                while pending_op:
                    pending_op.pop(0)()
            otp.release()

    nc.compile()
    return nc


_NC_CACHE = []


def _get_nc():
    if not _NC_CACHE:
        _NC_CACHE.append(build())
    return _NC_CACHE[0]


def make_in_maps(x, w_qkv, w_out):
    import ml_dtypes

    x2 = x.reshape(NTOK, C).astype(np.float32)
    # xTt[p, tci*KB + kb, j] = x[tci*512 + j, kb*128 + p]
    xTt = np.ascontiguousarray(
        x2.reshape(NTC, 512, KB, 128).transpose(3, 0, 2, 1)
        .reshape(128, NTC * KB, 512)).astype(ml_dtypes.bfloat16)
    scale = np.float32(1.0 / math.sqrt(D))

    inv = 1.0 / (10000.0 ** (np.arange(0, D, 2, dtype=np.float32) / D))
    pos = np.arange(T, dtype=np.float32)
    ang = pos[:, None] * inv[None, :]            # [T, 64]
    cosT = np.cos(ang).T.astype(np.float32)      # [64, T]
    sinT = np.sin(ang).T.astype(np.float32)
    cos2 = np.ascontiguousarray(np.vstack([cosT, cosT]))   # [128, T]
    sin2 = np.ascontiguousarray(np.vstack([sinT, sinT]))

    in_maps = []
    for c in range(NCORES):
        q = w_qkv[256 * c: 256 * (c + 1)] * scale
        k = w_qkv[C + 256 * c: C + 256 * (c + 1)]
        v = w_qkv[2 * C + 256 * c: 2 * C + 256 * (c + 1)]
        wl = np.concatenate([q, k, v], axis=0)       # [768, C]
        # wTt[p, kb, m] = wl[m, kb*128 + p]
        wTt = np.ascontiguousarray(
            wl.T.reshape(KB, 128, 3 * HPC * D).transpose(1, 0, 2)
        ).astype(ml_dtypes.bfloat16)
        woT = np.ascontiguousarray(
            w_out[:, 256 * c: 256 * (c + 1)].T).astype(ml_dtypes.bfloat16)
        in_maps.append({
            "xTt": xTt, "wTt": wTt, "woT": woT,
            "cos2": cos2, "sin2": sin2,
        })
    return in_maps


def run(x, w_qkv, w_out, trace=False):
    nc = _get_nc()
    in_maps = make_in_maps(x, w_qkv, w_out)
    res = run_bass_kernel_spmd(nc, in_maps, core_ids=list(range(NCORES)),
                               trace=trace)
    y = res.results[0]["out"].astype(np.float32)
    for i in range(1, NCORES):
        y = y + res.results[i]["out"].astype(np.float32)
    return y.reshape(B, T, C), res


def kernel(x, w_qkv, w_out):
    y, _ = run(x, w_qkv, w_out, trace=False)
    return y
